# revision 1
# baseline (speedup 1.0000x reference)
# Trainium2 Bass kernel: 2:4 structured activation pruning + Linear.
#
#   out = magnitude_prune_2of4(x.reshape(-1, 4096)) @ weight.T
#
# Sharding: data-parallel over the flattened token dim (16384 tokens ->
# 2048/core across 8 cores); weight replicated (host-transposed so the
# contraction dim lands on SBUF partitions). No collectives.
#
# Per-core pipeline, per 128-token tile (free dim split in 2 halves of 2048):
#   DMA x -> ACT |x| -> DVE pairwise min/max tree -> per-group-of-4 2nd-max
#   threshold (exact fp32) -> DVE drop-mask + predicated zero (in place)
#   -> PE 128x128 transposes (fp32, exact) -> ACT PSUM->SBUF copy w/ cast to
#   float32r -> PE matmul (float32r, full rate) accumulating over 32 d-chunks
#   -> ACT PSUM->SBUF -> DMA out.
import numpy as np

N_CORES = 8
BS, SEQ, D = 4, 4096, 4096
OUTF = 1024
TOK_TOTAL = BS * SEQ
TOK = TOK_TOTAL // N_CORES      # 2048 tokens per core
P = 128                         # SBUF partitions
NT = TOK // P                   # 16 token tiles per core
HALF = D // 2                   # 2048: free-dim half width
NCH = D // P                    # 32 d-chunks of 128
NCH_H = NCH // 2                # 16 d-chunks per half

_compiled = None
_custom_ops = None


def _register_custom_dve():
    # Fused DVE ops (registered into the runtime op table; compiled into the
    # per-NEFF DVE table): pairwise abs-max/abs-min, and the pruning select
    # out = |x| >= thr ? x : 0. Halves DVE work vs stock-op sequences.
    global _custom_ops
    if _custom_ops is not None:
        return _custom_ops
    from concourse import dve_ops as D
    from concourse.dve_spec import Spec, Src0, Src1, Zero, maxx, minn, select, lower
    from concourse.dve_uop import DveOpSpec

    def mk(name, body, reference):
        spec = Spec(body=body, reference=reference)
        shas = {}
        for ver in ("v3", "v4"):
            try:
                u = lower(spec, ver=ver)
                shas[ver] = DveOpSpec(name=name, opcode=1, uops=u,
                                      rd1_en=True).sha(ver)
            except Exception:
                if ver == "v3":
                    raise
        return D.DveOp(name=name, spec=spec, subdim=False, uops_sha=shas)

    absa = maxx(Src0, Zero - Src0)
    absb = maxx(Src1, Zero - Src1)
    ops = (
        mk("ABS_MAX2_ANT", maxx(absa, absb),
           lambda in0, in1: np.maximum(np.abs(in0), np.abs(in1))),
        mk("ABS_MIN2_ANT", minn(absa, absb),
           lambda in0, in1: np.minimum(np.abs(in0), np.abs(in1))),
        mk("PRUNE24_ANT", select(maxx(Src0, Zero - Src0) >= Src1, Src0, Zero),
           lambda in0, in1: np.where(np.abs(in0) >= in1, in0, 0.0)),
    )
    for op in ops:
        if op.name not in D._SUB_OPCODE_FOR_NAME:
            D.OPS.append(op)
            D.CUSTOM_DVE_SPECS[op.name] = op.spec
            D._SUB_OPCODE_FOR_NAME[op.name] = (
                D._CUSTOM_DVE_ROW_BASE + len(D._SUB_OPCODE_FOR_NAME))
    _custom_ops = ops
    return ops


def _build():
    import concourse.tile as tile
    import concourse.mybir as mybir
    from concourse import bacc
    from concourse.masks import make_identity

    ABS_MAX2, ABS_MIN2, PRUNE24 = _register_custom_dve()
    f32 = mybir.dt.float32
    f32r = mybir.dt.float32r
    Alu = mybir.AluOpType

    nc = bacc.Bacc("TRN2", target_bir_lowering=False, debug=False,
                   num_devices=N_CORES)
    xs_ap = nc.dram_tensor("xs", [TOK, D], f32, kind="ExternalInput").ap()
    wt_ap = nc.dram_tensor("wt", [D, OUTF], f32r, kind="ExternalInput").ap()
    o_ap = nc.dram_tensor("o", [TOK, OUTF], f32, kind="ExternalOutput").ap()

    with tile.TileContext(nc) as tc:
        with tc.tile_pool(name="wpool", bufs=1) as wpool, \
             tc.tile_pool(name="consts", bufs=1) as consts, \
             tc.tile_pool(name="xin", bufs=2) as xin, \
             tc.tile_pool(name="mwork", bufs=1) as mwork, \
             tc.tile_pool(name="xtp", bufs=2) as xtp, \
             tc.tile_pool(name="outp", bufs=1) as outp, \
             tc.tile_pool(name="pstr", bufs=4, space="PSUM") as pstr, \
             tc.tile_pool(name="pso", bufs=4, space="PSUM") as pso:

            # ---- constants ----
            # weight.T resident in SBUF: [d-in-chunk partitions, chunk, outf].
            # Loaded via the gpsimd DMA queue so the 16MB transfer doesn't
            # serialize ahead of the x-tile loads on the sync queue.
            ident = consts.tile([P, P], f32)
            make_identity(nc, ident)
            ident_r = consts.tile([P, P], f32r)
            nc.vector.tensor_copy(ident_r, ident)
            w_sb = wpool.tile([P, NCH, OUTF], f32r)
            for c in range(NCH):
                nc.gpsimd.dma_start(out=w_sb[:, c, :],
                                    in_=wt_ap[c * P:(c + 1) * P, :])

            def process_span(i, xspT, lo, w):
                # prune x[i-tile, lo:lo+w] and deposit transposed f32r chunks
                # into xspT[:, lo//P : (lo+w)//P, :]
                xh = xin.tile([P, w], f32, tag="xh", bufs=2,
                              padded_shape=[P, HALF])
                nc.sync.dma_start(out=xh, in_=xs_ap[i * P:(i + 1) * P,
                                                    lo:lo + w])
                # pairwise tree: thr = 2nd-largest |x| per group of 4
                x2 = xh.rearrange("p (g two) -> p g two", two=2)
                mx = mwork.tile([P, w // 2], f32, tag="mx",
                                padded_shape=[P, HALF // 2])
                mn = mwork.tile([P, w // 2], f32, tag="mn",
                                padded_shape=[P, HALF // 2])
                nc.vector._custom_dve(ABS_MAX2, out=mx,
                                      in0=x2[:, :, 0], in1=x2[:, :, 1])
                nc.vector._custom_dve(ABS_MIN2, out=mn,
                                      in0=x2[:, :, 0], in1=x2[:, :, 1])
                # compact in place: writes trail the strided reads
                mx2 = mx.rearrange("p (g two) -> p g two", two=2)
                mn2 = mn.rearrange("p (g two) -> p g two", two=2)
                mm = mx[:, :w // 4]
                nm = mn[:, :w // 4]
                nc.vector.tensor_tensor(mm, mx2[:, :, 0], mx2[:, :, 1], Alu.min)
                nc.vector.tensor_tensor(nm, mn2[:, :, 0], mn2[:, :, 1], Alu.max)
                thr = mm
                nc.vector.tensor_tensor(thr, mm, nm, Alu.max)
                # prune: xspr = |x| >= thr ? x : 0, rounded to float32r
                # so the PE transposes run at 1.5 cyc/row instead of 2
                thr_b = thr.unsqueeze(2).broadcast_to([P, w // 4, 4])
                xspr = mwork.tile([P, w], f32r, tag="xspr", bufs=2,
                                  padded_shape=[P, HALF])
                nc.vector._custom_dve(
                    PRUNE24,
                    out=xspr.rearrange("p (g four) -> p g four", four=4),
                    in0=xh.rearrange("p (g four) -> p g four", four=4),
                    in1=thr_b)
                # transpose chunks of [128,128] via PE, 4 per PSUM bank
                for b in range(w // P // 4):
                    ptr = pstr.tile([P, 4 * P], f32r)
                    for k in range(4):
                        cc = 4 * b + k
                        nc.tensor.transpose(ptr[:, k * P:(k + 1) * P],
                                            xspr[:, cc * P:(cc + 1) * P],
                                            ident_r)
                    c0 = lo // P + 4 * b
                    nc.scalar.copy(xspT[:, c0:c0 + 4, :], ptr)

            for i in range(NT):
                # float32r-rounded transposed pruned activations, [d, tok]
                xspT = xtp.tile([P, NCH, P], f32r)
                for lo in range(0, D, HALF):
                    process_span(i, xspT, lo, HALF)
                # matmul: psum[tok, outf-half] += xspT[c].T @ wT[c]
                for n in range(2):
                    pout = pso.tile([P, OUTF // 2], f32)
                    for c in range(NCH):
                        nc.tensor.matmul(pout,
                                         xspT[:, c, :],
                                         w_sb[:, c, n * 512:(n + 1) * 512],
                                         start=(c == 0), stop=(c == NCH - 1))
                    osb = outp.tile([P, OUTF // 2], f32)
                    nc.scalar.copy(osb, pout)
                    nc.sync.dma_start(
                        out=o_ap[i * P:(i + 1) * P, n * 512:(n + 1) * 512],
                        in_=osb)
    nc.compile()
    return nc


def _get_compiled():
    global _compiled
    if _compiled is None:
        _compiled = _build()
    return _compiled


def _fix_ties(x_flat):
    # The device keeps elements with |x| >= (2nd-largest |x| of the group).
    # On an exact fp32 tie |2nd|==|3rd| that keeps 3 elements, while the
    # reference (top_k, stable) keeps the lower-indexed 2. Pre-zero the
    # reference-dropped elements of tied groups so the device agrees; the
    # zeroed elements are dropped either way, so values are unaffected.
    g = np.abs(x_flat.reshape(-1, 4))
    m1 = np.maximum(g[:, 0], g[:, 1]); n1 = np.minimum(g[:, 0], g[:, 1])
    m2 = np.maximum(g[:, 2], g[:, 3]); n2 = np.minimum(g[:, 2], g[:, 3])
    thr = np.maximum(np.minimum(m1, m2), np.maximum(n1, n2))
    third = np.minimum(np.minimum(m1, m2), np.maximum(n1, n2))
    tied = np.flatnonzero(thr == third)
    if len(tied) == 0:
        return x_flat
    x_flat = x_flat.copy()
    gv = x_flat.reshape(-1, 4)
    for t in tied:
        row = gv[t]
        order = np.argsort(-np.abs(row), kind="stable")
        row[order[2:]] = 0.0
    return x_flat


def kernel(x: np.ndarray, weight: np.ndarray) -> np.ndarray:
    from concourse.bass_utils import run_bass_kernel_spmd

    nc = _get_compiled()
    x_flat = np.ascontiguousarray(x.reshape(TOK_TOTAL, D), dtype=np.float32)
    x_flat = _fix_ties(x_flat)
    wt = np.ascontiguousarray(weight.T, dtype=np.float32)
    in_maps = [{"xs": x_flat[c * TOK:(c + 1) * TOK], "wt": wt}
               for c in range(N_CORES)]
    res = run_bass_kernel_spmd(nc, in_maps, core_ids=list(range(N_CORES)))
    out = np.concatenate([res.results[c]["o"] for c in range(N_CORES)], axis=0)
    return out.reshape(BS, SEQ, OUTF)

